# revision 1
# baseline (speedup 1.0000x reference)
"""GATv2 2-layer GNN on 8 Trainium2 NeuronCores (Bass/Tile) — v2.

Strategy (dst-sharded edge parallelism, bf16 PE pipeline):
- Nodes dst-sharded: 6250/core packed into 49 chunks of 128 slots (LPT on edge
  counts, chunks relabeled by descending load so per-chunk tile counts align
  across cores). Per-chunk tile counts are baked into the program.
- Layer 1 is gather-free: the host pre-orders x in edge order; each tile's
  source features arrive as a bf16 [128in, 128e] lhsT and xj = x_srcT.T @ W1
  is computed on the PE. xi comes from a one-hot matmul against the chunk's
  projected features. All one-hot/scatter matmuls run in bf16.
- One-hots are built on DVE tensor_scalar (is_equal); attention logits via 4
  per-head fused scalar_tensor_tensor (mult + accum); softmax denominator via
  a 4-wide scatter matmul column block.
- Layer-1 output is normalized+ReLU'd per chunk, transposed on PE, projected
  through W2 inline, and written bf16 to per-slice buffers; 4 sliced
  AllGathers (pipelined behind the layer-1 chunk loop) build the replicated
  h2 table. Layer-2 xj comes from int16 pair-index dma_gathers (bf16 rows,
  queue_num rotated across Q7 pairs).
- Post-MP linears run inline in the layer-2 epilogue.
"""

import numpy as np

N = 50000
E = 800000
IN = 128
HC = 256
H = 4
C64 = 64
OUT = 64
SLOPE = 0.2
NCORES = 8
NPC = N // NCORES
CHUNKS = 49
P = 128
SHARD = CHUNKS * P
GSLOTS = NCORES * SHARD
PAD_DST = 255.0
NSLICES = 4
GB = 2  # tiles per group


def _slice_of_chunk(j):
    # chunk slices for the 4 pipelined AllGathers: [13, 12, 12, 12]
    sizes = [13, 12, 12, 12]
    k = 0
    acc = 0
    for k, s in enumerate(sizes):
        if j < acc + s:
            return k, acc, s
        acc += s
    raise AssertionError


SLICE_SIZES = [13, 12, 12, 12]
SLICE_STARTS = [0, 13, 25, 37]


def _pack_core(dst_local, n_nodes=NPC, chunks=CHUNKS):
    """LPT-pack nodes into chunks of <=128, then relabel by load desc."""
    deg = np.bincount(dst_local, minlength=n_nodes)
    order = np.argsort(-deg, kind="stable")
    bin_load = np.zeros(chunks, np.int64)
    bin_cnt = np.zeros(chunks, np.int32)
    bin_members = [[] for _ in range(chunks)]
    for v in order:
        cand = np.where(bin_cnt < P)[0]
        b = cand[np.argmin(bin_load[cand])]
        bin_members[b].append(v)
        bin_load[b] += deg[v]
        bin_cnt[b] += 1
    # relabel chunks by decreasing load so per-chunk maxima align across cores
    relabel = np.argsort(-bin_load, kind="stable")
    perm = np.full(chunks * P, -1, np.int64)
    for newb, oldb in enumerate(relabel):
        for k, v in enumerate(bin_members[oldb]):
            perm[newb * P + k] = v
    return perm


def _wrap_idx(flat):
    n = flat.shape[0]
    w = flat.reshape(n // 16, 16).T.astype(np.int16)
    return np.tile(w, (8, 1)).copy()


def prepare(inputs):
    import ml_dtypes
    bf16 = ml_dtypes.bfloat16
    x = np.asarray(inputs["x"], np.float32)
    ei = np.asarray(inputs["edge_index"]).astype(np.int64)
    src, dst = ei[0], ei[1]
    owner = dst // NPC
    dst_local = dst - owner * NPC

    perms = []
    for c in range(NCORES):
        m = owner == c
        perms.append(_pack_core(dst_local[m]))

    # slice-major global slot layout:
    # g(core, chunk j, pos p) = (SLICE_STARTS[k]*NCORES + core*SLICE_SIZES[k]
    #                            + (j - j0)) * 128 + p
    def gslot(core, j, p):
        k, j0, s = _slice_of_chunk(j)
        return (SLICE_STARTS[k] * NCORES + core * s + (j - j0)) * P + p

    pos_of = np.empty(N, np.int64)
    own_row = np.empty((NCORES, SHARD), np.int64)  # node id per (core, j*128+p)
    for c in range(NCORES):
        perm = perms[c]
        for j in range(CHUNKS):
            for p in range(P):
                v = perm[j * P + p]
                if v >= 0:
                    pos_of[v + c * NPC] = gslot(c, j, p)
                own_row[c, j * P + p] = v  # -1 for empty

    gsrc = pos_of[src]
    gdst_core = owner
    chunk_of_edge = np.empty(E, np.int64)
    slot_of_edge = np.empty(E, np.int64)
    # recover (chunk, slot) of dst on its owner core
    inv = {}
    for c in range(NCORES):
        perm = perms[c]
        loc = np.full(NPC, -1, np.int64)
        valid = perm >= 0
        loc[perm[valid]] = np.nonzero(valid)[0]
        m = owner == c
        lp = loc[dst_local[m]]
        chunk_of_edge[m] = lp // P
        slot_of_edge[m] = lp % P
    par = (gsrc & 1).astype(np.int64)

    ev_lists = [[[] for _ in range(CHUNKS)] for _ in range(NCORES)]
    od_lists = [[[] for _ in range(CHUNKS)] for _ in range(NCORES)]
    for e in range(E):
        tgt = ev_lists if par[e] == 0 else od_lists
        tgt[gdst_core[e]][chunk_of_edge[e]].append(e)

    tcnt_ev = tuple(
        max(1, max((len(ev_lists[c][j]) + P - 1) // P for c in range(NCORES)))
        for j in range(CHUNKS))
    tcnt_od = tuple(
        max(1, max((len(od_lists[c][j]) + P - 1) // P for c in range(NCORES)))
        for j in range(CHUNKS))
    T = [a + b for a, b in zip(tcnt_ev, tcnt_od)]
    off_t = np.concatenate([[0], np.cumsum(T)]).astype(np.int64)
    off_ev = np.concatenate([[0], np.cumsum(tcnt_ev)]).astype(np.int64)
    off_od = np.concatenate([[0], np.cumsum(tcnt_od)]).astype(np.int64)
    TOT_T = int(off_t[-1])
    TOT_EV = int(off_ev[-1])
    TOT_OD = int(off_od[-1])

    xsrcT = np.zeros((NCORES, P, TOT_T * P), bf16)      # [in, tile-major edges]
    ev_idx = np.zeros((NCORES, P, TOT_EV * 8), np.int16)
    od_idx = np.zeros((NCORES, P, TOT_OD * 8), np.int16)
    dst_meta = np.full((NCORES, P, TOT_T), PAD_DST, np.float32)
    dst_meta_rep = np.full((NCORES, TOT_T * P), PAD_DST, bf16)

    xT = np.ascontiguousarray(x.T.astype(bf16))  # [IN, N]
    for c in range(NCORES):
        for j in range(CHUNKS):
            for edges, tcnt, toff, idx_arr, ioff in (
                (ev_lists[c][j], tcnt_ev[j], off_t[j], ev_idx, off_ev[j]),
                (od_lists[c][j], tcnt_od[j], off_t[j] + tcnt_ev[j], od_idx,
                 off_od[j]),
            ):
                ne = len(edges)
                earr = np.asarray(edges, np.int64)
                flat = np.zeros(tcnt * P, np.int64)
                if ne:
                    flat[:ne] = gsrc[earr] >> 1
                idx_arr[c, :, ioff * 8:(ioff + tcnt) * 8] = _wrap_idx(flat)
                if ne:
                    cols = toff * P + np.arange(ne)
                    xsrcT[c][:, cols] = xT[:, src[earr]]
                    tt = toff + np.arange(ne) // P
                    pp = np.arange(ne) % P
                    dst_meta[c, pp, tt] = slot_of_edge[earr]
                    dst_meta_rep[c, tt * P + pp] = slot_of_edge[earr]

    x_ownT = np.zeros((NCORES, P, SHARD), bf16)
    for c in range(NCORES):
        valid = own_row[c] >= 0
        x_ownT[c][:, valid] = xT[:, own_row[c][valid] + c * NPC]

    W1T = np.ascontiguousarray(np.asarray(inputs["W1"], np.float32).T).astype(bf16)
    W2T = np.ascontiguousarray(np.asarray(inputs["W2"], np.float32).T).astype(bf16)
    W3T = np.ascontiguousarray(np.asarray(inputs["W3"], np.float32).T).astype(bf16)
    W4T = np.ascontiguousarray(np.asarray(inputs["W4"], np.float32).T).astype(bf16)
    att1 = np.asarray(inputs["att1"], np.float32).reshape(1, HC)
    att2 = np.asarray(inputs["att2"], np.float32).reshape(1, HC)
    att1_rep = np.tile(att1, (P, 1)).astype(bf16)
    att2_rep = np.tile(att2, (P, 1)).astype(bf16)
    b1 = np.asarray(inputs["b1"], np.float32)
    b2 = np.asarray(inputs["b2"], np.float32)
    b3 = np.asarray(inputs["b3"], np.float32)
    b4 = np.asarray(inputs["b4"], np.float32)
    has_b1 = bool(np.any(b1 != 0.0))
    b1_rep = np.tile(b1.reshape(1, HC), (P, 1)).astype(np.float32)

    in_maps = []
    for c in range(NCORES):
        in_maps.append({
            "xsrcT": np.ascontiguousarray(xsrcT[c]),
            "x_ownT": np.ascontiguousarray(x_ownT[c]),
            "ev_idx": np.ascontiguousarray(ev_idx[c]),
            "od_idx": np.ascontiguousarray(od_idx[c]),
            "dst_meta": np.ascontiguousarray(dst_meta[c]),
            "dst_meta_rep": np.ascontiguousarray(
                np.broadcast_to(dst_meta_rep[c][None, :], (P, TOT_T * P))),
            "W1T": W1T, "W2T": W2T, "W3T": W3T, "W4T": W4T,
            "b1x2_row": (2.0 * b1).reshape(1, HC).astype(bf16),
            "b2_row": b2.reshape(1, HC).astype(bf16),
            "b2x2_row": (2.0 * b2).reshape(1, HC).astype(bf16),
            "b3_row": b3.reshape(1, OUT).astype(bf16),
            "b4_row": b4.reshape(1, OUT).astype(bf16),
            "b1_rep": b1_rep,
            "att1_rep": att1_rep, "att2_rep": att2_rep,
        })
    meta = dict(tcnt_ev=tcnt_ev, tcnt_od=tcnt_od, has_b1=has_b1)
    return in_maps, perms, meta


# ------------------------------------------------------------- device build

def build(tcnt_ev, tcnt_od, has_b1=False, part_bcast=True):
    import concourse.bacc as bacc
    import concourse.mybir as mybir
    import concourse.tile as tile
    from concourse.masks import make_identity

    dt = mybir.dt
    AF = mybir.ActivationFunctionType
    ALU = mybir.AluOpType
    AX = mybir.AxisListType

    T = [a + b for a, b in zip(tcnt_ev, tcnt_od)]
    off_t = np.concatenate([[0], np.cumsum(T)]).astype(np.int64)
    off_ev = np.concatenate([[0], np.cumsum(tcnt_ev)]).astype(np.int64)
    off_od = np.concatenate([[0], np.cumsum(tcnt_od)]).astype(np.int64)
    TOT_T = int(off_t[-1])
    TOT_EV = int(off_ev[-1])
    TOT_OD = int(off_od[-1])
    TMAX = max(T)

    nc = bacc.Bacc("TRN2", target_bir_lowering=False, debug=False,
                   num_devices=NCORES, num_swdge_queues=4)

    xsrcT = nc.dram_tensor("xsrcT", [P, TOT_T * P], dt.bfloat16, kind="ExternalInput")
    x_ownT = nc.dram_tensor("x_ownT", [P, SHARD], dt.bfloat16, kind="ExternalInput")
    ev_idx = nc.dram_tensor("ev_idx", [P, TOT_EV * 8], dt.int16, kind="ExternalInput")
    od_idx = nc.dram_tensor("od_idx", [P, TOT_OD * 8], dt.int16, kind="ExternalInput")
    dst_meta = nc.dram_tensor("dst_meta", [P, TOT_T], dt.float32, kind="ExternalInput")
    dst_meta_rep = nc.dram_tensor("dst_meta_rep", [P, TOT_T * P], dt.bfloat16,
                                  kind="ExternalInput")
    W1T = nc.dram_tensor("W1T", [IN, HC], dt.bfloat16, kind="ExternalInput")
    W2T = nc.dram_tensor("W2T", [HC, HC], dt.bfloat16, kind="ExternalInput")
    W3T = nc.dram_tensor("W3T", [HC, OUT], dt.bfloat16, kind="ExternalInput")
    W4T = nc.dram_tensor("W4T", [OUT, OUT], dt.bfloat16, kind="ExternalInput")
    b1x2_row = nc.dram_tensor("b1x2_row", [1, HC], dt.bfloat16, kind="ExternalInput")
    b2_row = nc.dram_tensor("b2_row", [1, HC], dt.bfloat16, kind="ExternalInput")
    b2x2_row = nc.dram_tensor("b2x2_row", [1, HC], dt.bfloat16, kind="ExternalInput")
    b3_row = nc.dram_tensor("b3_row", [1, OUT], dt.bfloat16, kind="ExternalInput")
    b4_row = nc.dram_tensor("b4_row", [1, OUT], dt.bfloat16, kind="ExternalInput")
    b1_rep = nc.dram_tensor("b1_rep", [P, HC], dt.float32, kind="ExternalInput")
    att1_rep = nc.dram_tensor("att1_rep", [P, HC], dt.bfloat16, kind="ExternalInput")
    att2_rep = nc.dram_tensor("att2_rep", [P, HC], dt.bfloat16, kind="ExternalInput")
    y_shard = nc.dram_tensor("y_shard", [SHARD, OUT], dt.float32, kind="ExternalOutput")

    h2in = [nc.dram_tensor(f"h2in_{k}", [SLICE_SIZES[k] * P, HC], dt.bfloat16)
            for k in range(NSLICES)]
    h2_full = nc.dram_tensor("h2_full", [GSLOTS, HC], dt.bfloat16,
                             addr_space="Shared")
    rg = [list(range(NCORES))]

    with tile.TileContext(nc, num_cores=NCORES) as tc:
        with tc.tile_pool(name="const", bufs=1) as constp:
            identf = constp.tile([P, P], dt.float32)
            make_identity(nc, identf[:])
            iota_row = constp.tile([P, P], dt.bfloat16)
            nc.gpsimd.iota(iota_row[:], pattern=[[1, P]], base=0,
                           channel_multiplier=0,
                           allow_small_or_imprecise_dtypes=True)
            iota_col = constp.tile([P, 1], dt.float32)
            nc.gpsimd.iota(iota_col[:], pattern=[[0, 1]], base=0,
                           channel_multiplier=1,
                           allow_small_or_imprecise_dtypes=True)
            ident = constp.tile([P, P], dt.bfloat16)
            nc.vector.tensor_scalar(out=ident[:], in0=iota_row[:],
                                    scalar1=iota_col[:, 0:1], scalar2=None,
                                    op0=ALU.is_equal)
            ones_row = constp.tile([1, P], dt.bfloat16)
            nc.gpsimd.memset(ones_row[:], 1.0)
            iota_crep = constp.tile([P, GB * P], dt.bfloat16)
            nc.gpsimd.iota(iota_crep[:], pattern=[[0, GB * P]], base=0,
                           channel_multiplier=1,
                           allow_small_or_imprecise_dtypes=True)

            att_t = {}
            for l, t_ in ((1, att1_rep), (2, att2_rep)):
                at = constp.tile([P, GB, HC], dt.bfloat16, name=f"att{l}")
                for g in range(GB):
                    nc.sync.dma_start(out=at[:, g, :], in_=t_[:])
                att_t[l] = at
            bias_t = {}
            for name, t_, w in (("b1x2", b1x2_row, HC), ("b2", b2_row, HC),
                                ("b2x2", b2x2_row, HC),
                                ("b3", b3_row, OUT), ("b4", b4_row, OUT)):
                bt = constp.tile([1, w], dt.bfloat16, name=f"bias_{name}")
                nc.sync.dma_start(out=bt[:], in_=t_[:])
                bias_t[name] = bt
            b1r_t = None
            if has_b1:
                b1r_t = constp.tile([P, HC], dt.float32, name="b1rep")
                nc.sync.dma_start(out=b1r_t[:], in_=b1_rep[:])
            wtile = {}
            for name, t_, kk, w in (("w1", W1T, IN, HC),
                                    ("w2lo", W2T[0:P, :], P, HC),
                                    ("w2hi", W2T[P:2 * P, :], P, HC),
                                    ("w3lo", W3T[0:P, :], P, OUT),
                                    ("w3hi", W3T[P:2 * P, :], P, OUT),
                                    ("w4", W4T, OUT, OUT)):
                wt = constp.tile([kk, w], dt.bfloat16, name=f"w_{name}")
                nc.sync.dma_start(out=wt[:], in_=t_ if name not in ("w1", "w4") else t_[:])
                wtile[name] = wt
            xot = constp.tile([P, SHARD], dt.bfloat16, name="xot")
            nc.sync.dma_start(out=xot[:], in_=x_ownT[:])

            def edge_layer(layer):
                att_tile = att_t[layer]
                pairs = h2_full[:].rearrange("(a b) d -> a (b d)", b=2)
                with (
                    tc.tile_pool(name="chio", bufs=2) as chio,
                    tc.tile_pool(name="xin", bufs=2) as xin,
                    tc.tile_pool(name="work", bufs=2) as work,
                    tc.tile_pool(name="gps", bufs=2, space="PSUM") as gps,
                    tc.tile_pool(name="eps", bufs=1, space="PSUM") as eps,
                    tc.tile_pool(name="sps", bufs=1, space="PSUM") as sps,
                ):
                    for j in range(CHUNKS):
                        tj = T[j]
                        tev, tod = tcnt_ev[j], tcnt_od[j]
                        k, j0, sk = _slice_of_chunk(j)

                        # --- per-chunk feature table (projected, +bias) ---
                        hps = sps.tile([P, HC], dt.float32, tag="pmm")
                        if layer == 1:
                            nc.tensor.matmul(out=hps[:],
                                             lhsT=xot[:, j * P:(j + 1) * P],
                                             rhs=wtile["w1"][:],
                                             start=True, stop=False)
                            nc.tensor.matmul(out=hps[:], lhsT=ones_row[:],
                                             rhs=bias_t["b1x2"][:],
                                             start=False, stop=True)
                            hck = work.tile([P, HC], dt.bfloat16, tag="hck")
                            nc.scalar.activation(hck[:], hps[:], AF.Copy)
                        else:
                            hck = work.tile([P, HC], dt.bfloat16, tag="hck")
                            nc.sync.dma_start(
                                out=hck[:],
                                in_=h2in[k][(j - j0) * P:(j - j0 + 1) * P, :])

                        dmt = chio.tile([P, TMAX], dt.float32, tag="dmt")
                        nc.sync.dma_start(out=dmt[:, 0:tj],
                                          in_=dst_meta[:, off_t[j]:off_t[j] + tj])
                        dmtr = chio.tile([P, TMAX * P], dt.bfloat16, tag="dmtr")
                        nc.sync.dma_start(
                            out=dmtr[:, 0:tj * P],
                            in_=dst_meta_rep[:, off_t[j] * P:(off_t[j] + tj) * P])

                        if layer == 1:
                            xsr = xin.tile([P, TMAX * P], dt.bfloat16, tag="xsr")
                            nc.sync.dma_start(
                                out=xsr[:, 0:tj * P],
                                in_=xsrcT[:, off_t[j] * P:(off_t[j] + tj) * P])
                        else:
                            evi = chio.tile([P, max(tcnt_ev) * 8],
                                            dt.int16, tag="evi")
                            nc.sync.dma_start(
                                out=evi[:, 0:tev * 8],
                                in_=ev_idx[:, off_ev[j] * 8:off_ev[j + 1] * 8])
                            odi = chio.tile([P, max(tcnt_od) * 8],
                                            dt.int16, tag="odi")
                            nc.sync.dma_start(
                                out=odi[:, 0:tod * 8],
                                in_=od_idx[:, off_od[j] * 8:off_od[j + 1] * 8])
                            xj_ev = xin.tile([P, max(tcnt_ev), HC], dt.bfloat16,
                                             tag="xjev")
                            xj_od = xin.tile([P, max(tcnt_od), HC], dt.bfloat16,
                                             tag="xjod")
                            nc.gpsimd.dma_gather(
                                out_ap=xj_ev[:, 0:tev, :], in_ap=pairs[:, 0:HC],
                                idxs_ap=evi[:, 0:tev * 8], num_idxs=tev * P,
                                num_idxs_reg=tev * P, elem_size=HC,
                                elem_step=2 * HC, single_packet=False,
                                queue_num=1 + (2 * j) % 3)
                            nc.gpsimd.dma_gather(
                                out_ap=xj_od[:, 0:tod, :], in_ap=pairs[:, HC:2 * HC],
                                idxs_ap=odi[:, 0:tod * 8], num_idxs=tod * P,
                                num_idxs_reg=tod * P, elem_size=HC,
                                elem_step=2 * HC, single_packet=False,
                                queue_num=1 + (2 * j + 1) % 3)

                        msgden = eps.tile([P, HC + 4], dt.float32, tag="msgden",
                                          bufs=1)
                        groups = []
                        for base, tcnt, parity in ((0, tev, 0), (tev, tod, 1)):
                            t0 = 0
                            while t0 < tcnt:
                                gb = min(GB, tcnt - t0)
                                groups.append((base, t0, gb, parity))
                                t0 += gb

                        first = True
                        ngroups = len(groups)
                        for gi, (base, t0, gb, parity) in enumerate(groups):
                            glob0 = base + t0  # tile index within chunk
                            # one-hot builds
                            s_T = work.tile([P, GB * P], dt.bfloat16, tag="s_T")
                            s_en = work.tile([P, GB * P], dt.bfloat16, tag="s_en")
                            nc.vector.tensor_tensor(
                                out=s_T[:, 0:gb * P],
                                in0=dmtr[:, glob0 * P:(glob0 + gb) * P],
                                in1=iota_crep[:, 0:gb * P], op=ALU.is_equal)
                            nc.vector.tensor_tensor(
                                out=s_en[:, 0:gb * P].rearrange(
                                    "p (g e) -> p g e", g=gb),
                                in0=iota_row[:].rearrange(
                                    "p (o e) -> p o e", o=1).to_broadcast(
                                    [P, gb, P]),
                                in1=dmt[:, glob0:glob0 + gb].to_broadcast(
                                    [P, gb, P]),
                                op=ALU.is_equal)

                            ps_z = gps.tile([P, GB, HC], dt.float32, tag="ps_z")
                            if layer == 1:
                                ps_xj = gps.tile([P, GB, HC], dt.float32,
                                                 tag="ps_xj", bufs=2)
                                for i in range(gb):
                                    ti = glob0 + i
                                    xw = xsr[:, ti * P:(ti + 1) * P]
                                    nc.tensor.matmul(out=ps_xj[:, i, :], lhsT=xw,
                                                     rhs=wtile["w1"][:],
                                                     start=True, stop=True)
                                    nc.tensor.matmul(out=ps_z[:, i, :], lhsT=xw,
                                                     rhs=wtile["w1"][:],
                                                     start=True, stop=False)
                                    nc.tensor.matmul(
                                        out=ps_z[:, i, :],
                                        lhsT=s_T[:, i * P:(i + 1) * P],
                                        rhs=hck[:], start=False, stop=True)
                                xj_src = ps_xj
                                xj_off = 0
                            else:
                                pool = xj_ev if parity == 0 else xj_od
                                for i in range(gb):
                                    nc.tensor.matmul(
                                        out=ps_z[:, i, :],
                                        lhsT=s_T[:, i * P:(i + 1) * P],
                                        rhs=hck[:], start=True, stop=False)
                                nc.tensor.matmul(
                                    out=ps_z[:, 0:gb, :].rearrange("p a c -> p (a c)"),
                                    lhsT=ident[:],
                                    rhs=pool[:, t0:t0 + gb, :].rearrange(
                                        "p a c -> p (a c)"),
                                    start=False, stop=True)
                                xj_src = pool
                                xj_off = t0

                            s_b = work.tile([P, GB, HC], dt.bfloat16, tag="s_b")
                            nc.scalar.activation(
                                s_b[:, 0:gb, :].rearrange("p a c -> p (a c)"),
                                ps_z[:, 0:gb, :].rearrange("p a c -> p (a c)"),
                                AF.Prelu, alpha=SLOPE)

                            t_b = work.tile([P, GB, HC], dt.bfloat16, tag="t_b")
                            nc.vector.tensor_tensor(
                                out=t_b[:, 0:gb, :].rearrange("p a c -> p (a c)"),
                                in0=s_b[:, 0:gb, :].rearrange("p a c -> p (a c)"),
                                in1=att_tile[:, 0:gb, :].rearrange(
                                    "p a c -> p (a c)"),
                                op=ALU.mult)
                            alph = work.tile([P, GB * H], dt.float32, tag="alph")
                            nc.vector.tensor_reduce(
                                out=alph[:, 0:gb * H].rearrange(
                                    "p (a h) -> p a h", h=H),
                                in_=t_b[:, 0:gb, :].rearrange(
                                    "p a (h c) -> p a h c", h=H),
                                axis=AX.X, op=ALU.add)
                            msg = work.tile([P, GB, HC + 4], dt.bfloat16,
                                            tag="msg")
                            nc.scalar.activation(
                                msg[:, 0:gb, HC:HC + 4],
                                alph[:, 0:gb * H].rearrange(
                                    "p (a h) -> p a h", h=H), AF.Exp)
                            nc.vector.tensor_tensor(
                                out=msg[:, 0:gb, 0:HC].rearrange(
                                    "p a (h c) -> p a h c", h=H),
                                in0=xj_src[:, xj_off:xj_off + gb, :].rearrange(
                                    "p a (h c) -> p a h c", h=H),
                                in1=msg[:, 0:gb, HC:HC + 4].to_broadcast(
                                    [P, gb, H, C64]),
                                op=ALU.mult)
                            for i in range(gb):
                                last = (gi == ngroups - 1) and (i == gb - 1)
                                nc.tensor.matmul(
                                    out=msgden[:],
                                    lhsT=s_en[:, i * P:(i + 1) * P],
                                    rhs=msg[:, i, :], start=first, stop=last)
                                first = False

                        # ---------------- epilogue ----------------
                        den = work.tile([P, H], dt.float32, tag="den")
                        nc.vector.tensor_scalar(
                            out=den[:], in0=msgden[:, HC:HC + 4], scalar1=1e-20,
                            scalar2=None, op0=ALU.max)
                        rden = work.tile([P, H], dt.float32, tag="rden")
                        nc.vector.reciprocal(rden[:], den[:])
                        orl = work.tile([P, HC], dt.bfloat16, tag="orl")
                        if layer == 1 and has_b1:
                            tmp = work.tile([P, HC], dt.float32, tag="tmpb")
                            for h in range(H):
                                nc.vector.scalar_tensor_tensor(
                                    out=tmp[:, h * C64:(h + 1) * C64],
                                    in0=msgden[:, h * C64:(h + 1) * C64],
                                    scalar=rden[:, h:h + 1],
                                    in1=b1r_t[:, h * C64:(h + 1) * C64],
                                    op0=ALU.mult, op1=ALU.add)
                            nc.scalar.activation(orl[:], tmp[:], AF.Relu)
                        else:
                            for h in range(H):
                                nc.scalar.activation(
                                    orl[:, h * C64:(h + 1) * C64],
                                    msgden[:, h * C64:(h + 1) * C64],
                                    AF.Relu, scale=rden[:, h:h + 1])

                        trs = []
                        trpt = sps.tile([P, 2 * P], dt.bfloat16, tag="trp")
                        for half in range(2):
                            trp = trpt[:, half * P:(half + 1) * P]
                            nc.tensor.transpose(
                                out=trp, in_=orl[:, half * P:(half + 1) * P],
                                identity=ident[:])
                            tr = work.tile([P, P], dt.bfloat16, tag=f"trs{half}")
                            nc.scalar.activation(tr[:], trp, AF.Copy)
                            trs.append(tr)

                        if layer == 1:
                            # inline phase B: h2 = relu1 @ W2 + b2
                            h2ps = sps.tile([P, HC], dt.float32, tag="pmm")
                            nc.tensor.matmul(out=h2ps[:], lhsT=trs[0][:],
                                             rhs=wtile["w2lo"][:],
                                             start=True, stop=False)
                            nc.tensor.matmul(out=h2ps[:], lhsT=trs[1][:],
                                             rhs=wtile["w2hi"][:],
                                             start=False, stop=False)
                            nc.tensor.matmul(out=h2ps[:], lhsT=ones_row[:],
                                             rhs=bias_t["b2"][:],
                                             start=False, stop=True)
                            h2b = work.tile([P, HC], dt.bfloat16, tag="h2b")
                            nc.scalar.activation(h2b[:], h2ps[:], AF.Copy)
                            nc.sync.dma_start(
                                out=h2in[k][(j - j0) * P:(j - j0 + 1) * P, :],
                                in_=h2b[:])
                            if j - j0 == sk - 1:
                                nc.gpsimd.collective_compute(
                                    "AllGather", mybir.AluOpType.bypass,
                                    replica_groups=rg,
                                    ins=[h2in[k].ap().opt()],
                                    outs=[h2_full[
                                        SLICE_STARTS[k] * NCORES * P:
                                        (SLICE_STARTS[k] + SLICE_SIZES[k])
                                        * NCORES * P, :].opt()])
                        else:
                            # inline phase C: y = (relu2 @ W3 + b3) @ W4 + b4
                            pmm = sps.tile([P, HC], dt.float32, tag="pmm")
                            ps3 = pmm[:, 0:OUT]
                            nc.tensor.matmul(out=ps3, lhsT=trs[0][:],
                                             rhs=wtile["w3lo"][:],
                                             start=True, stop=False)
                            nc.tensor.matmul(out=ps3, lhsT=trs[1][:],
                                             rhs=wtile["w3hi"][:],
                                             start=False, stop=False)
                            nc.tensor.matmul(out=ps3, lhsT=ones_row[:],
                                             rhs=bias_t["b3"][:],
                                             start=False, stop=True)
                            h3 = work.tile([P, OUT], dt.bfloat16, tag="h3")
                            nc.scalar.activation(h3[:], ps3, AF.Copy)
                            h3tp = trpt[0:OUT, 0:P]
                            nc.tensor.transpose(out=h3tp, in_=h3[:],
                                                identity=ident[:])
                            h3t = work.tile([OUT, P], dt.bfloat16, tag="h3t")
                            nc.scalar.activation(h3t[:], h3tp, AF.Copy)
                            ps4 = pmm[:, OUT:2 * OUT]
                            nc.tensor.matmul(out=ps4, lhsT=h3t[:],
                                             rhs=wtile["w4"][:],
                                             start=True, stop=False)
                            nc.tensor.matmul(out=ps4, lhsT=ones_row[:],
                                             rhs=bias_t["b4"][:],
                                             start=False, stop=True)
                            yt = work.tile([P, OUT], dt.float32, tag="yt")
                            nc.scalar.activation(yt[:], ps4, AF.Copy)
                            nc.sync.dma_start(
                                out=y_shard[j * P:(j + 1) * P, :], in_=yt[:])

            edge_layer(1)
            edge_layer(2)

    nc.compile()
    return nc


# ----------------------------------------------------------------- kernel()

_CACHE = {}


def kernel(**inputs):
    from concourse.bass_utils import run_bass_kernel_spmd

    in_maps, perms, meta = prepare(inputs)
    key = (meta["tcnt_ev"], meta["tcnt_od"], meta["has_b1"])
    if key not in _CACHE:
        _CACHE[key] = build(meta["tcnt_ev"], meta["tcnt_od"], meta["has_b1"])
    nc = _CACHE[key]
    res = run_bass_kernel_spmd(nc, in_maps, core_ids=list(range(NCORES)))
    out = np.zeros((N, OUT), np.float32)
    for c in range(NCORES):
        ys = res.results[c]["y_shard"]
        valid = perms[c] >= 0
        out[perms[c][valid] + c * NPC] = ys[valid]
    return out


if __name__ == "__main__":
    import jax
    import reference
    cpu = jax.devices("cpu")[0]
    with jax.default_device(cpu):
        inputs = {k: np.asarray(v) for k, v in reference.setup_inputs().items()}
        exp = np.asarray(reference.reference(**inputs))
    got = kernel(**inputs)
    rel = np.linalg.norm(got - exp) / np.linalg.norm(exp)
    print("Relative error:", rel)

